# revision 34
# baseline (speedup 1.0000x reference)
"""GAT layer (N=4096, F=64, H=8, D=8) on 8 Trainium2 NeuronCores.

Row-parallel sharding: core c owns queries q0=512*c .. q0+512. Each core
reads the full X (replicated) and the transposed adjacency slice
A_T[j, i] = A[q0+i, j] (host-sliced, cast to bf16 -- exact for a 0/1 mask).

Math: with per-node logits a_s[i,h], a_n[j,h] and s = a_s+a_n,
  exp(leaky_relu(s)) = E1_i*F1_j  if s>=0   (E1=exp(a_s), F1=exp(a_n))
                     = E2_i*F2_j  if s<0    (E2=exp(.2 a_s), F2=exp(.2 a_n))
so no exp over the [H,N,N] tensor is ever needed.  Per head only the {0,1}
mask W1 = A ⊙ [s>=0] is materialized, and the aggregation is PE matmuls:
  U1 = G1^T@W1, U2 = G2^T@W1, UA2 = G2^T@A   (G_k[j] = F_k[j]*[feats_j | 1])
  out = E1*U1 + E2*(UA2-U2);  numerators / denominator; +bias; relu.

Mask production is split across three engines per 16 key tiles:
  - 6 tiles on DVE: tensor_scalar is_ge (4x mode) + one batched
    tensor_tensor mult against A (2x mode),
  - 5 tiles on ACT: relu(1e30*u + 1e30*v) -> {0, huge}, then one batched
    DVE min against A,
  - 5 tiles on Pool: fused scalar_tensor_tensor (is_ge, mult).
The UA2 (unmasked) matmul pass runs between heads 3 and 4 when the PE is
warm, and the E-table/combine work is deferred to the tail.
"""

import sys

sys.path.insert(0, "/opt/trn_rl_repo")

import ml_dtypes
import numpy as np

N, F, H, D = 4096, 64, 8, 8
HD = H * D  # 64
NCORES = 8
Q = N // NCORES  # 512 queries per core
NT = N // 128  # 32 key tiles
QT = Q // 128  # 4 query tiles
GB = 8  # key-tile group size for feats/G-table phase

_CACHED_NC = None


def build_bass(do_compile=True):
    import concourse.bacc as bacc
    import concourse.mybir as mybir
    from concourse.masks import make_identity
    from concourse.tile import TileContext

    f32 = mybir.dt.float32
    bf16 = mybir.dt.bfloat16
    Alu = mybir.AluOpType
    Act = mybir.ActivationFunctionType

    nc = bacc.Bacc()

    XT_d = nc.declare_dram_parameter("XT", [F, N], bf16, isOutput=False)
    XqT_d = nc.declare_dram_parameter("XqT", [F, Q], f32, isOutput=False)
    W_d = nc.declare_dram_parameter("W", [F, HD], f32, isOutput=False)
    Wbf_d = nc.declare_dram_parameter("Wbf", [F, HD], bf16, isOutput=False)
    attS_d = nc.declare_dram_parameter("attS", [1, HD], f32, isOutput=False)
    attN_d = nc.declare_dram_parameter("attN", [1, HD], f32, isOutput=False)
    bias_d = nc.declare_dram_parameter("bias", [HD, 1], f32, isOutput=False)
    # SEL64[h, h*8+d] = 1 (row -> 8-row group expander)
    sel_d = nc.declare_dram_parameter("SEL64", [H, HD], f32, isOutput=False)
    # DELTA[k, h*128 + p] = (k == h) (row-h selector for K=8 broadcast matmuls)
    delta_d = nc.declare_dram_parameter("DELTA", [H, H * 128], bf16, isOutput=False)
    AT_d = nc.declare_dram_parameter("AT", [N, Q], bf16, isOutput=False)
    out_d = nc.declare_dram_parameter("out", [HD, Q], f32, isOutput=True)

    with TileContext(nc) as tc:
        with (
            tc.tile_pool(name="big", bufs=1) as big,
            tc.tile_pool(name="tmp", bufs=2) as tmp,
            tc.tile_pool(name="w1p", bufs=4) as w1p,
            tc.tile_pool(name="ps_small", bufs=2, space="PSUM") as ps_small,
            tc.tile_pool(name="ps_acc", bufs=1, space="PSUM") as ps_acc,
        ):
            # ---- persistent SBUF tensors
            A_sb = big.tile([128, NT, Q], bf16)  # A_T tiles, [j_tile, i]
            GG = big.tile([128, NT, H * 18], bf16)  # per head: 9 G1 | 9 G2 cols
            G2a = big.tile([128, NT, 72], bf16)  # contiguous all-heads G2
            a_sbB = big.tile([128, H, Q], bf16)  # a_s bcast over partitions
            na_all = big.tile([128, NT, H], f32)  # -a_n
            na_big = big.tile([128, NT, H], f32)  # +1e30 * a_n
            feats_all = big.tile([128, NT, HD], bf16)
            F1_all = big.tile([128, NT, H], f32)
            F2_all = big.tile([128, NT, H], f32)
            XT_sb = big.tile([F, N], bf16)
            XqT_sb = big.tile([F, Q], f32)
            W_sb = big.tile([F, HD], f32)
            Wbf_sb = big.tile([F, HD], bf16)
            attS_b = big.tile([128, HD], f32)
            attN_b = big.tile([128, HD], bf16)
            bias_c = big.tile([HD, 1], f32)
            sel_sb = big.tile([H, HD], f32)
            delta_sb = big.tile([H, H * 128], bf16)
            a_sT = big.tile([H, Q], f32)
            a_sT_bf = big.tile([H, Q], bf16)
            E1T = big.tile([H, Q], f32)
            E2T = big.tile([H, Q], f32)
            E1n = big.tile([HD, Q], f32)
            E2n = big.tile([HD, Q], f32)
            U1n = big.tile([HD, Q], f32)
            U2n = big.tile([HD, Q], f32)
            UAn = big.tile([HD, Q], f32)
            U1den = big.tile([36, Q], f32)
            U2den = big.tile([36, Q], f32)
            UAden = big.tile([36, Q], f32)
            E1Td = big.tile([36, Q], f32)
            E2Td = big.tile([36, Q], f32)
            selD = big.tile([36, 4 * D], f32)
            OUT_T = big.tile([HD, Q], f32)

            # ---- input DMAs (small phase-0-critical tensors FIRST)
            nc.sync.dma_start(out=W_sb[:], in_=W_d[:])
            nc.sync.dma_start(out=XqT_sb[:], in_=XqT_d[:])
            nc.sync.dma_start(out=XT_sb[:], in_=XT_d[:])
            nc.sync.dma_start(out=Wbf_sb[:], in_=Wbf_d[:])
            nc.sync.dma_start(out=bias_c[:], in_=bias_d[:])
            nc.sync.dma_start(out=sel_sb[:], in_=sel_d[:])
            nc.sync.dma_start(out=delta_sb[:], in_=delta_d[:])
            att_row = tmp.tile([1, HD], f32, tag="attrow")
            nc.sync.dma_start(out=att_row[:], in_=attS_d[:])
            att_row2 = tmp.tile([1, HD], f32, tag="attrow")
            nc.sync.dma_start(out=att_row2[:], in_=attN_d[:])
            for nt in range(NT):
                nc.sync.dma_start(
                    out=A_sb[:, nt, :], in_=AT_d[nt * 128 : (nt + 1) * 128, :]
                )

            # broadcast helper: ones row for K=1 "broadcast" matmuls
            ones_f = big.tile([1, 128], f32)
            nc.gpsimd.memset(ones_f[:], 1.0)
            ps_b = ps_small.tile([128, HD], f32, tag="psB", bufs=1)
            nc.tensor.matmul(ps_b[:], ones_f[:], att_row[:], start=True, stop=True)
            nc.scalar.copy(attS_b[:], ps_b[:])
            ps_b2 = ps_small.tile([128, HD], f32, tag="psB", bufs=1)
            nc.tensor.matmul(ps_b2[:], ones_f[:], att_row2[:], start=True, stop=True)
            nc.scalar.copy(attN_b[:], ps_b2[:])

            ident = big.tile([128, 128], f32)
            make_identity(nc, ident[:])

            # ---- a_s for this core's queries, transposed; a_sbB broadcast
            psFq = ps_small.tile([128, QT * HD], f32, tag="psB", bufs=1)
            for qt in range(QT):
                nc.tensor.matmul(
                    psFq[:, qt * HD : (qt + 1) * HD],
                    XqT_sb[:, qt * 128 : (qt + 1) * 128],
                    W_sb[:],
                    start=True,
                    stop=True,
                )
            prodq = tmp.tile([128, QT, HD], f32, tag="prodq", bufs=1)
            nc.vector.tensor_tensor(
                out=prodq[:],
                in0=psFq.rearrange("p (a b) -> p a b", b=HD),
                in1=attS_b[:, None, :].broadcast_to([128, QT, HD]),
                op=Alu.mult,
            )
            a_s_t = tmp.tile([128, QT, H], f32, tag="a_s", bufs=1)
            nc.vector.tensor_reduce(
                out=a_s_t[:],
                in_=prodq.rearrange("p a (h d) -> p a h d", d=D),
                axis=mybir.AxisListType.X,
                op=Alu.add,
            )
            for qt in range(QT):
                psT = ps_small.tile([H, 128], f32, tag="psT", bufs=1)
                nc.tensor.transpose(psT[:], a_s_t[:, qt, :], ident[:])
                nc.scalar.copy(a_sT[:, qt * 128 : (qt + 1) * 128], psT[:])

            nc.vector.tensor_copy(out=a_sT_bf[:], in_=a_sT[:])
            # a_sbB[:, h, :] = row h of a_sT broadcast to 128 partitions
            for h in range(H):
                ps_bc = ps_small.tile([128, Q], f32, tag="psBC", bufs=1)
                nc.tensor.matmul(
                    ps_bc[:],
                    delta_sb[:, h * 128 : (h + 1) * 128],
                    a_sT_bf[:],
                    start=True,
                    stop=True,
                )
                if h % 2 == 0:
                    nc.scalar.copy(a_sbB[:, h, :], ps_bc[:])
                else:
                    nc.vector.tensor_copy(out=a_sbB[:, h, :], in_=ps_bc[:])

            # E tables (needed only by the combine; PE/ACT have slack here)
            nc.scalar.activation(E1T[:], a_sT[:], Act.Exp)
            nc.scalar.activation(E2T[:], a_sT[:], Act.Exp, scale=0.2)
            ps_e = ps_small.tile([HD, Q], f32, tag="psBC", bufs=1)
            nc.tensor.matmul(ps_e[:], sel_sb[:], E1T[:], start=True, stop=True)
            nc.vector.tensor_copy(out=E1n[:], in_=ps_e[:])
            ps_e2 = ps_small.tile([HD, Q], f32, tag="psBC", bufs=1)
            nc.tensor.matmul(ps_e2[:], sel_sb[:], E2T[:], start=True, stop=True)
            nc.scalar.copy(E2n[:], ps_e2[:])
            nc.sync.dma_start(out=selD[0:4, :], in_=sel_sb[0:4, 0 : 4 * D])
            nc.sync.dma_start(out=selD[32:36, :], in_=sel_sb[0:4, 0 : 4 * D])
            nc.sync.dma_start(out=E1Td[0:4, :], in_=E1T[0:4, :])
            nc.sync.dma_start(out=E1Td[32:36, :], in_=E1T[4:8, :])
            nc.sync.dma_start(out=E2Td[0:4, :], in_=E2T[0:4, :])
            nc.sync.dma_start(out=E2Td[32:36, :], in_=E2T[4:8, :])

            # ---- phase 0: feats, a_n, F1/F2, G tables (all 4096 nodes)
            gg_all = GG.rearrange("p a (h u) -> p a h u", u=18)
            g2_all = G2a.rearrange("p a (h u) -> p a h u", u=9)
            feats_v = feats_all.rearrange("p a (h d) -> p a h d", d=D)
            for g in range(0, NT, GB):
                sl = slice(g, g + GB)
                psF = ps_small.tile([128, GB * HD], f32, tag="psF")
                for t in range(GB):
                    nt = g + t
                    nc.tensor.matmul(
                        psF[:, t * HD : (t + 1) * HD],
                        XT_sb[:, nt * 128 : (nt + 1) * 128],
                        Wbf_sb[:],
                        start=True,
                        stop=True,
                    )
                nc.scalar.copy(
                    feats_all[:, sl, :].rearrange("p a b -> p (a b)"),
                    psF[:],
                )
                prod = tmp.tile([128, GB, HD], bf16, tag="prod")
                nc.vector.tensor_tensor(
                    out=prod[:],
                    in0=feats_all[:, sl, :],
                    in1=attN_b[:, None, :].broadcast_to([128, GB, HD]),
                    op=Alu.mult,
                )
                nc.vector.tensor_reduce(
                    out=na_all[:, sl, :],
                    in_=prod.rearrange("p a (h d) -> p a h d", d=D),
                    axis=mybir.AxisListType.X,
                    op=Alu.add,
                    negate=True,
                )
                nc.vector.tensor_scalar_mul(
                    out=na_big[:, sl, :].rearrange("p a b -> p (a b)"),
                    in0=na_all[:, sl, :].rearrange("p a b -> p (a b)"),
                    scalar1=-1e30,
                )
                nc.scalar.activation(
                    F1_all[:, sl, :].rearrange("p a b -> p (a b)"),
                    na_all[:, sl, :].rearrange("p a b -> p (a b)"),
                    Act.Exp,
                    scale=-1.0,
                )
                nc.scalar.activation(
                    F2_all[:, sl, :].rearrange("p a b -> p (a b)"),
                    na_all[:, sl, :].rearrange("p a b -> p (a b)"),
                    Act.Exp,
                    scale=-0.2,
                )
                nc.vector.tensor_tensor(
                    out=gg_all[:, sl, :, 0:8],
                    in0=feats_v[:, sl],
                    in1=F1_all[:, sl, :, None].broadcast_to([128, GB, H, D]),
                    op=Alu.mult,
                )
                nc.vector.tensor_copy(
                    out=gg_all[:, sl, :, 8:9], in_=F1_all[:, sl, :, None]
                )
                nc.vector.tensor_tensor(
                    out=g2_all[:, sl, :, 0:8],
                    in0=feats_v[:, sl],
                    in1=F2_all[:, sl, :, None].broadcast_to([128, GB, H, D]),
                    op=Alu.mult,
                )
                nc.vector.tensor_copy(
                    out=g2_all[:, sl, :, 8:9], in_=F2_all[:, sl, :, None]
                )
                nc.vector.tensor_copy(
                    out=gg_all[:, sl, :, 9:18], in_=g2_all[:, sl]
                )

            # ---- phase 2: per head, W1 = [s>=0] ⊙ A; U = GG_h^T @ W1
            # per 16-tile block: 6 DVE / 5 ACT / 5 Pool mask tiles
            def emit_head(h):
                psU = ps_acc.tile([18, Q], f32, tag="psU", bufs=2)
                for b in range(0, NT, 16):
                    # DVE tiles b..b+6: sigma in {0,1} via 4x tensor_scalar,
                    # then one in-place batched mult against A
                    w1D = w1p.tile([128, 7, Q], bf16, tag="w1D")
                    for t in range(7):
                        nt = b + t
                        nc.vector.tensor_scalar(
                            out=w1D[:, t, :],
                            in0=a_sbB[:, h, :],
                            scalar1=na_all[:, nt, h : h + 1],
                            scalar2=None,
                            op0=Alu.is_ge,
                        )
                    nc.vector.tensor_tensor(
                        out=w1D[:],
                        in0=w1D[:],
                        in1=A_sb[:, b : b + 7, :],
                        op=Alu.mult,
                    )
                    # ACT tiles b+7..b+15: sigma in {0, huge} via relu,
                    # then one in-place batched DVE min against A
                    w1A = w1p.tile([128, 9, Q], bf16, tag="w1A")
                    for t in range(9):
                        nt = b + 7 + t
                        nc.scalar.activation(
                            w1A[:, t, :],
                            a_sbB[:, h, :],
                            Act.Relu,
                            scale=1e30,
                            bias=na_big[:, nt, h : h + 1],
                        )
                    nc.vector.tensor_tensor(
                        out=w1A[:],
                        in0=w1A[:],
                        in1=A_sb[:, b + 7 : b + 16, :],
                        op=Alu.min,
                    )
                    # matmuls in nt order
                    for t in range(16):
                        nt = b + t
                        if t < 7:
                            rhs = w1D[:, t, :]
                        else:
                            rhs = w1A[:, t - 7, :]
                        nc.tensor.matmul(
                            psU[:],
                            GG[:, nt, h * 18 : (h + 1) * 18],
                            rhs,
                            start=(nt == 0),
                            stop=(nt == NT - 1),
                        )
                U_sb = tmp.tile([18, Q], f32, tag="U", bufs=3)
                if h % 2 == 0:
                    nc.scalar.copy(U_sb[:], psU[:])
                else:
                    nc.vector.tensor_copy(out=U_sb[:], in_=psU[:])
                nc.sync.dma_start(
                    out=U1n[h * D : (h + 1) * D, :], in_=U_sb[0:8, :]
                )
                hr = h if h < 4 else h + 28
                nc.sync.dma_start(out=U1den[hr : hr + 1, :], in_=U_sb[8:9, :])
                nc.sync.dma_start(
                    out=U2n[h * D : (h + 1) * D, :], in_=U_sb[9:17, :]
                )
                nc.sync.dma_start(out=U2den[hr : hr + 1, :], in_=U_sb[17:18, :])

            def emit_ua2():
                # UA2 = G2^T @ A_T -> [72, Q], strided G2 columns from GG
                psA = ps_acc.tile([72, Q], f32, tag="psA")
                for nt in range(NT):
                    nc.tensor.matmul(
                        psA[:],
                        G2a[:, nt, :],
                        A_sb[:, nt, :],
                        start=(nt == 0),
                        stop=(nt == NT - 1),
                    )
                UA_sb = tmp.tile([72, Q], f32, tag="UA", bufs=1)
                nc.vector.tensor_copy(out=UA_sb[:], in_=psA[:])
                for h in range(H):
                    nc.sync.dma_start(
                        out=UAn[h * D : (h + 1) * D, :],
                        in_=UA_sb[h * 9 : h * 9 + 8, :],
                    )
                    hr = h if h < 4 else h + 28
                    nc.sync.dma_start(
                        out=UAden[hr : hr + 1, :],
                        in_=UA_sb[h * 9 + 8 : h * 9 + 9, :],
                    )

            # numerator half-combine for heads [h0, h0+4): RNn rows
            RNn = big.tile([HD, Q], f32)
            TnF = big.tile([HD, Q], f32)
            M1F = big.tile([HD, Q], f32)

            def emit_num_half(h0):
                rs = slice(h0 * D, h0 * D + 4 * D)  # 32 numerator rows
                nc.vector.tensor_sub(out=TnF[rs, :], in0=UAn[rs, :], in1=U2n[rs, :])
                nc.vector.tensor_mul(out=M1F[rs, :], in0=E1n[rs, :], in1=U1n[rs, :])
                nc.vector.tensor_mul(out=TnF[rs, :], in0=E2n[rs, :], in1=TnF[rs, :])
                nc.vector.tensor_add(out=RNn[rs, :], in0=M1F[rs, :], in1=TnF[rs, :])

            Tdf = big.tile([36, Q], f32)
            M1df = big.tile([36, Q], f32)
            RNdf = big.tile([36, Q], f32)
            rcpd = big.tile([36, Q], f32)
            scF = big.tile([HD, Q], f32)

            def emit_den_finish(h0):
                p0 = 0 if h0 == 0 else 32
                sl = slice(p0, p0 + 4)
                rs = slice(h0 * D, h0 * D + 4 * D)
                nc.vector.tensor_sub(out=Tdf[sl, :], in0=UAden[sl, :], in1=U2den[sl, :])
                nc.vector.tensor_mul(out=M1df[sl, :], in0=E1Td[sl, :], in1=U1den[sl, :])
                nc.vector.tensor_mul(out=Tdf[sl, :], in0=E2Td[sl, :], in1=Tdf[sl, :])
                nc.vector.tensor_add(out=RNdf[sl, :], in0=M1df[sl, :], in1=Tdf[sl, :])
                nc.vector.reciprocal(rcpd[sl, :], RNdf[sl, :])
                ps_rc = ps_small.tile([4 * D, Q], f32, tag="psBC", bufs=1)
                nc.tensor.matmul(
                    ps_rc[:], selD[sl, :], rcpd[sl, :],
                    start=True, stop=True,
                )
                nc.vector.tensor_mul(out=scF[rs, :], in0=RNn[rs, :], in1=ps_rc[:])
                nc.scalar.activation(
                    OUT_T[rs, :], scF[rs, :], Act.Relu, bias=bias_c[rs, :]
                )
                nc.sync.dma_start(out=out_d[rs, :], in_=OUT_T[rs, :])

            for h in range(4):
                emit_head(h)
            emit_ua2()
            emit_num_half(0)
            emit_den_finish(0)
            for h in range(4, H):
                emit_head(h)
            emit_num_half(4)
            emit_den_finish(4)

    if do_compile:
        nc.compile()
    return nc


def _get_nc():
    global _CACHED_NC
    if _CACHED_NC is None:
        _CACHED_NC = build_bass()
    return _CACHED_NC


def make_in_maps(X, A, W, att_self, att_neigh, bias):
    X = np.asarray(X, np.float32)
    A = np.asarray(A, np.float32)
    W = np.asarray(W, np.float32)
    att_self = np.asarray(att_self, np.float32)
    att_neigh = np.asarray(att_neigh, np.float32)
    bias = np.asarray(bias, np.float32)

    XT = np.ascontiguousarray(X.T).astype(ml_dtypes.bfloat16)
    Wbf = W.astype(ml_dtypes.bfloat16)
    attS = np.ascontiguousarray(att_self.reshape(1, HD))
    attN = np.ascontiguousarray(att_neigh.reshape(1, HD))
    bias_c = np.ascontiguousarray(bias.reshape(HD, 1))
    sel = np.zeros((H, HD), np.float32)
    for h in range(H):
        sel[h, h * D : (h + 1) * D] = 1.0
    delta = np.zeros((H, H * 128), np.float32)
    for h in range(H):
        delta[h, h * 128 : (h + 1) * 128] = 1.0
    delta = delta.astype(ml_dtypes.bfloat16)
    in_maps = []
    for c in range(NCORES):
        q0 = c * Q
        AT = np.ascontiguousarray(A[q0 : q0 + Q, :].T).astype(ml_dtypes.bfloat16)
        XqT = np.ascontiguousarray(X[q0 : q0 + Q, :].T)
        in_maps.append(
            {
                "XT": XT,
                "XqT": XqT,
                "W": W,
                "Wbf": Wbf,
                "attS": attS,
                "attN": attN,
                "bias": bias_c,
                "SEL64": sel,
                "DELTA": delta,
                "AT": AT,
            }
        )
    return in_maps


def kernel(X, A, W, att_self, att_neigh, bias, _trace=False, _tmpdir=None):
    from concourse.bass_utils import run_bass_kernel_spmd

    nc = _get_nc()
    in_maps = make_in_maps(X, A, W, att_self, att_neigh, bias)
    res = run_bass_kernel_spmd(
        nc,
        in_maps,
        core_ids=list(range(NCORES)),
        trace=_trace,
        tmpdir=_tmpdir,
    )
    out = np.empty((N, HD), np.float32)
    for c in range(NCORES):
        out[c * Q : (c + 1) * Q, :] = res.results[c]["out"].T
    if _trace:
        return out, res
    return out


# revision 35
# speedup vs baseline: 1.0381x; 1.0381x over previous
"""GAT layer (N=4096, F=64, H=8, D=8) on 8 Trainium2 NeuronCores.

Row-parallel sharding: core c owns queries q0=512*c .. q0+512. Each core
reads the full X (replicated) and the transposed adjacency slice
A_T[j, i] = A[q0+i, j] (host-sliced, cast to bf16 -- exact for a 0/1 mask).

Math: with per-node logits a_s[i,h], a_n[j,h] and s = a_s+a_n,
  exp(leaky_relu(s)) = E1_i*F1_j  if s>=0   (E1=exp(a_s), F1=exp(a_n))
                     = E2_i*F2_j  if s<0    (E2=exp(.2 a_s), F2=exp(.2 a_n))
so no exp over the [H,N,N] tensor is ever needed.  Per head only the {0,1}
mask W1 = A ⊙ [s>=0] is materialized, and the aggregation is PE matmuls:
  U1 = G1^T@W1, U2 = G2^T@W1, UA2 = G2^T@A   (G_k[j] = F_k[j]*[feats_j | 1])
  out = E1*U1 + E2*(UA2-U2);  numerators / denominator; +bias; relu.

Mask production is split across three engines per 16 key tiles:
  - 6 tiles on DVE: tensor_scalar is_ge (4x mode) + one batched
    tensor_tensor mult against A (2x mode),
  - 5 tiles on ACT: relu(1e30*u + 1e30*v) -> {0, huge}, then one batched
    DVE min against A,
  - 5 tiles on Pool: fused scalar_tensor_tensor (is_ge, mult).
The UA2 (unmasked) matmul pass runs between heads 3 and 4 when the PE is
warm, and the E-table/combine work is deferred to the tail.
"""

import sys

sys.path.insert(0, "/opt/trn_rl_repo")

import ml_dtypes
import numpy as np

N, F, H, D = 4096, 64, 8, 8
HD = H * D  # 64
NCORES = 8
Q = N // NCORES  # 512 queries per core
NT = N // 128  # 32 key tiles
QT = Q // 128  # 4 query tiles
GB = 8  # key-tile group size for feats/G-table phase

_CACHED_NC = None


def build_bass(do_compile=True):
    import concourse.bacc as bacc
    import concourse.mybir as mybir
    from concourse.masks import make_identity
    from concourse.tile import TileContext

    f32 = mybir.dt.float32
    bf16 = mybir.dt.bfloat16
    Alu = mybir.AluOpType
    Act = mybir.ActivationFunctionType

    nc = bacc.Bacc()

    XT_d = nc.declare_dram_parameter("XT", [F, N], bf16, isOutput=False)
    XqT_d = nc.declare_dram_parameter("XqT", [F, Q], f32, isOutput=False)
    W_d = nc.declare_dram_parameter("W", [F, HD], f32, isOutput=False)
    Wbf_d = nc.declare_dram_parameter("Wbf", [F, HD], bf16, isOutput=False)
    attS_d = nc.declare_dram_parameter("attS", [1, HD], f32, isOutput=False)
    attN_d = nc.declare_dram_parameter("attN", [1, HD], f32, isOutput=False)
    bias_d = nc.declare_dram_parameter("bias", [HD, 1], f32, isOutput=False)
    # SEL64[h, h*8+d] = 1 (row -> 8-row group expander)
    sel_d = nc.declare_dram_parameter("SEL64", [H, HD], f32, isOutput=False)
    # DELTA[k, h*128 + p] = (k == h) (row-h selector for K=8 broadcast matmuls)
    delta_d = nc.declare_dram_parameter("DELTA", [H, H * 128], bf16, isOutput=False)
    AT_d = nc.declare_dram_parameter("AT", [N, Q], bf16, isOutput=False)
    out_d = nc.declare_dram_parameter("out", [HD, Q], f32, isOutput=True)

    with TileContext(nc) as tc:
        with (
            tc.tile_pool(name="big", bufs=1) as big,
            tc.tile_pool(name="tmp", bufs=2) as tmp,
            tc.tile_pool(name="w1p", bufs=4) as w1p,
            tc.tile_pool(name="ps_small", bufs=2, space="PSUM") as ps_small,
            tc.tile_pool(name="ps_acc", bufs=1, space="PSUM") as ps_acc,
        ):
            # ---- persistent SBUF tensors
            A_sb = big.tile([128, NT, Q], bf16)  # A_T tiles, [j_tile, i]
            GG = big.tile([128, NT, H * 18], bf16)  # per head: 9 G1 | 9 G2 cols
            G2a = big.tile([128, NT, 72], bf16)  # contiguous all-heads G2
            a_sbB = big.tile([128, H, Q], bf16)  # a_s bcast over partitions
            na_all = big.tile([128, NT, H], f32)  # -a_n
            na_big = big.tile([128, NT, H], f32)  # +1e30 * a_n
            feats_all = big.tile([128, NT, HD], bf16)
            F1_all = big.tile([128, NT, H], f32)
            F2_all = big.tile([128, NT, H], f32)
            XT_sb = big.tile([F, N], bf16)
            XqT_sb = big.tile([F, Q], f32)
            W_sb = big.tile([F, HD], f32)
            Wbf_sb = big.tile([F, HD], bf16)
            attS_b = big.tile([128, HD], f32)
            attN_b = big.tile([128, HD], bf16)
            bias_c = big.tile([HD, 1], f32)
            sel_sb = big.tile([H, HD], f32)
            delta_sb = big.tile([H, H * 128], bf16)
            a_sT = big.tile([H, Q], f32)
            a_sT_bf = big.tile([H, Q], bf16)
            E1T = big.tile([H, Q], f32)
            E2T = big.tile([H, Q], f32)
            E1n = big.tile([HD, Q], f32)
            E2n = big.tile([HD, Q], f32)
            U1n = big.tile([HD, Q], f32)
            U2n = big.tile([HD, Q], f32)
            UAn = big.tile([HD, Q], f32)
            U1den = big.tile([H, Q], f32)
            U2den = big.tile([H, Q], f32)
            UAden = big.tile([H, Q], f32)
            OUT_T = big.tile([HD, Q], f32)

            # ---- input DMAs (small phase-0-critical tensors FIRST)
            nc.sync.dma_start(out=W_sb[:], in_=W_d[:])
            nc.sync.dma_start(out=XqT_sb[:], in_=XqT_d[:])
            nc.sync.dma_start(out=XT_sb[:], in_=XT_d[:])
            nc.sync.dma_start(out=Wbf_sb[:], in_=Wbf_d[:])
            nc.sync.dma_start(out=bias_c[:], in_=bias_d[:])
            nc.sync.dma_start(out=sel_sb[:], in_=sel_d[:])
            nc.sync.dma_start(out=delta_sb[:], in_=delta_d[:])
            att_row = tmp.tile([1, HD], f32, tag="attrow")
            nc.sync.dma_start(out=att_row[:], in_=attS_d[:])
            att_row2 = tmp.tile([1, HD], f32, tag="attrow")
            nc.sync.dma_start(out=att_row2[:], in_=attN_d[:])
            for nt in range(NT):
                nc.sync.dma_start(
                    out=A_sb[:, nt, :], in_=AT_d[nt * 128 : (nt + 1) * 128, :]
                )

            # broadcast helper: ones row for K=1 "broadcast" matmuls
            ones_f = big.tile([1, 128], f32)
            nc.gpsimd.memset(ones_f[:], 1.0)
            ps_b = ps_small.tile([128, HD], f32, tag="psB", bufs=1)
            nc.tensor.matmul(ps_b[:], ones_f[:], att_row[:], start=True, stop=True)
            nc.scalar.copy(attS_b[:], ps_b[:])
            ps_b2 = ps_small.tile([128, HD], f32, tag="psB", bufs=1)
            nc.tensor.matmul(ps_b2[:], ones_f[:], att_row2[:], start=True, stop=True)
            nc.scalar.copy(attN_b[:], ps_b2[:])

            ident = big.tile([128, 128], f32)
            make_identity(nc, ident[:])

            # ---- a_s for this core's queries, transposed; a_sbB broadcast
            psFq = ps_small.tile([128, QT * HD], f32, tag="psB", bufs=1)
            for qt in range(QT):
                nc.tensor.matmul(
                    psFq[:, qt * HD : (qt + 1) * HD],
                    XqT_sb[:, qt * 128 : (qt + 1) * 128],
                    W_sb[:],
                    start=True,
                    stop=True,
                )
            prodq = tmp.tile([128, QT, HD], f32, tag="prodq", bufs=1)
            nc.vector.tensor_tensor(
                out=prodq[:],
                in0=psFq.rearrange("p (a b) -> p a b", b=HD),
                in1=attS_b[:, None, :].broadcast_to([128, QT, HD]),
                op=Alu.mult,
            )
            a_s_t = tmp.tile([128, QT, H], f32, tag="a_s", bufs=1)
            nc.vector.tensor_reduce(
                out=a_s_t[:],
                in_=prodq.rearrange("p a (h d) -> p a h d", d=D),
                axis=mybir.AxisListType.X,
                op=Alu.add,
            )
            for qt in range(QT):
                psT = ps_small.tile([H, 128], f32, tag="psT", bufs=1)
                nc.tensor.transpose(psT[:], a_s_t[:, qt, :], ident[:])
                nc.scalar.copy(a_sT[:, qt * 128 : (qt + 1) * 128], psT[:])

            nc.vector.tensor_copy(out=a_sT_bf[:], in_=a_sT[:])
            # a_sbB[:, h, :] = row h of a_sT broadcast to 128 partitions
            for h in range(H):
                ps_bc = ps_small.tile([128, Q], f32, tag="psBC", bufs=1)
                nc.tensor.matmul(
                    ps_bc[:],
                    delta_sb[:, h * 128 : (h + 1) * 128],
                    a_sT_bf[:],
                    start=True,
                    stop=True,
                )
                if h % 2 == 0:
                    nc.scalar.copy(a_sbB[:, h, :], ps_bc[:])
                else:
                    nc.vector.tensor_copy(out=a_sbB[:, h, :], in_=ps_bc[:])

            # E tables (needed only by the combine; PE/ACT have slack here)
            nc.scalar.activation(E1T[:], a_sT[:], Act.Exp)
            nc.scalar.activation(E2T[:], a_sT[:], Act.Exp, scale=0.2)
            ps_e = ps_small.tile([HD, Q], f32, tag="psBC", bufs=1)
            nc.tensor.matmul(ps_e[:], sel_sb[:], E1T[:], start=True, stop=True)
            nc.vector.tensor_copy(out=E1n[:], in_=ps_e[:])
            ps_e2 = ps_small.tile([HD, Q], f32, tag="psBC", bufs=1)
            nc.tensor.matmul(ps_e2[:], sel_sb[:], E2T[:], start=True, stop=True)
            nc.scalar.copy(E2n[:], ps_e2[:])

            # ---- phase 0: feats, a_n, F1/F2, G tables (all 4096 nodes)
            gg_all = GG.rearrange("p a (h u) -> p a h u", u=18)
            g2_all = G2a.rearrange("p a (h u) -> p a h u", u=9)
            feats_v = feats_all.rearrange("p a (h d) -> p a h d", d=D)
            for g in range(0, NT, GB):
                sl = slice(g, g + GB)
                psF = ps_small.tile([128, GB * HD], f32, tag="psF")
                for t in range(GB):
                    nt = g + t
                    nc.tensor.matmul(
                        psF[:, t * HD : (t + 1) * HD],
                        XT_sb[:, nt * 128 : (nt + 1) * 128],
                        Wbf_sb[:],
                        start=True,
                        stop=True,
                    )
                nc.scalar.copy(
                    feats_all[:, sl, :].rearrange("p a b -> p (a b)"),
                    psF[:],
                )
                prod = tmp.tile([128, GB, HD], bf16, tag="prod")
                nc.vector.tensor_tensor(
                    out=prod[:],
                    in0=feats_all[:, sl, :],
                    in1=attN_b[:, None, :].broadcast_to([128, GB, HD]),
                    op=Alu.mult,
                )
                nc.vector.tensor_reduce(
                    out=na_all[:, sl, :],
                    in_=prod.rearrange("p a (h d) -> p a h d", d=D),
                    axis=mybir.AxisListType.X,
                    op=Alu.add,
                    negate=True,
                )
                nc.vector.tensor_scalar_mul(
                    out=na_big[:, sl, :].rearrange("p a b -> p (a b)"),
                    in0=na_all[:, sl, :].rearrange("p a b -> p (a b)"),
                    scalar1=-1e30,
                )
                nc.scalar.activation(
                    F1_all[:, sl, :].rearrange("p a b -> p (a b)"),
                    na_all[:, sl, :].rearrange("p a b -> p (a b)"),
                    Act.Exp,
                    scale=-1.0,
                )
                nc.scalar.activation(
                    F2_all[:, sl, :].rearrange("p a b -> p (a b)"),
                    na_all[:, sl, :].rearrange("p a b -> p (a b)"),
                    Act.Exp,
                    scale=-0.2,
                )
                nc.vector.tensor_tensor(
                    out=gg_all[:, sl, :, 0:8],
                    in0=feats_v[:, sl],
                    in1=F1_all[:, sl, :, None].broadcast_to([128, GB, H, D]),
                    op=Alu.mult,
                )
                nc.vector.tensor_copy(
                    out=gg_all[:, sl, :, 8:9], in_=F1_all[:, sl, :, None]
                )
                nc.vector.tensor_tensor(
                    out=g2_all[:, sl, :, 0:8],
                    in0=feats_v[:, sl],
                    in1=F2_all[:, sl, :, None].broadcast_to([128, GB, H, D]),
                    op=Alu.mult,
                )
                nc.vector.tensor_copy(
                    out=g2_all[:, sl, :, 8:9], in_=F2_all[:, sl, :, None]
                )
                nc.vector.tensor_copy(
                    out=gg_all[:, sl, :, 9:18], in_=g2_all[:, sl]
                )

            # ---- phase 2: per head, W1 = [s>=0] ⊙ A; U = GG_h^T @ W1
            # per 16-tile block: 6 DVE / 5 ACT / 5 Pool mask tiles
            def emit_head(h):
                psU = ps_acc.tile([18, Q], f32, tag="psU", bufs=2)
                for b in range(0, NT, 16):
                    # DVE tiles b..b+6: sigma in {0,1} via 4x tensor_scalar,
                    # then one in-place batched mult against A
                    w1D = w1p.tile([128, 7, Q], bf16, tag="w1D")
                    for t in range(7):
                        nt = b + t
                        nc.vector.tensor_scalar(
                            out=w1D[:, t, :],
                            in0=a_sbB[:, h, :],
                            scalar1=na_all[:, nt, h : h + 1],
                            scalar2=None,
                            op0=Alu.is_ge,
                        )
                    nc.vector.tensor_tensor(
                        out=w1D[:],
                        in0=w1D[:],
                        in1=A_sb[:, b : b + 7, :],
                        op=Alu.mult,
                    )
                    # ACT tiles b+7..b+15: sigma in {0, huge} via relu,
                    # then one in-place batched DVE min against A
                    w1A = w1p.tile([128, 9, Q], bf16, tag="w1A")
                    for t in range(9):
                        nt = b + 7 + t
                        nc.scalar.activation(
                            w1A[:, t, :],
                            a_sbB[:, h, :],
                            Act.Relu,
                            scale=1e30,
                            bias=na_big[:, nt, h : h + 1],
                        )
                    nc.vector.tensor_tensor(
                        out=w1A[:],
                        in0=w1A[:],
                        in1=A_sb[:, b + 7 : b + 16, :],
                        op=Alu.min,
                    )
                    # matmuls in nt order
                    for t in range(16):
                        nt = b + t
                        if t < 7:
                            rhs = w1D[:, t, :]
                        else:
                            rhs = w1A[:, t - 7, :]
                        nc.tensor.matmul(
                            psU[:],
                            GG[:, nt, h * 18 : (h + 1) * 18],
                            rhs,
                            start=(nt == 0),
                            stop=(nt == NT - 1),
                        )
                U_sb = tmp.tile([18, Q], f32, tag="U", bufs=3)
                if h % 2 == 0:
                    nc.scalar.copy(U_sb[:], psU[:])
                else:
                    nc.vector.tensor_copy(out=U_sb[:], in_=psU[:])
                nc.sync.dma_start(
                    out=U1n[h * D : (h + 1) * D, :], in_=U_sb[0:8, :]
                )
                nc.sync.dma_start(out=U1den[h : h + 1, :], in_=U_sb[8:9, :])
                nc.sync.dma_start(
                    out=U2n[h * D : (h + 1) * D, :], in_=U_sb[9:17, :]
                )
                nc.sync.dma_start(out=U2den[h : h + 1, :], in_=U_sb[17:18, :])

            def emit_ua2():
                # UA2 = G2^T @ A_T -> [72, Q], strided G2 columns from GG
                psA = ps_acc.tile([72, Q], f32, tag="psA")
                for nt in range(NT):
                    nc.tensor.matmul(
                        psA[:],
                        G2a[:, nt, :],
                        A_sb[:, nt, :],
                        start=(nt == 0),
                        stop=(nt == NT - 1),
                    )
                UA_sb = tmp.tile([72, Q], f32, tag="UA", bufs=1)
                nc.vector.tensor_copy(out=UA_sb[:], in_=psA[:])
                for h in range(H):
                    nc.sync.dma_start(
                        out=UAn[h * D : (h + 1) * D, :],
                        in_=UA_sb[h * 9 : h * 9 + 8, :],
                    )
                    nc.sync.dma_start(
                        out=UAden[h : h + 1, :],
                        in_=UA_sb[h * 9 + 8 : h * 9 + 9, :],
                    )

            # numerator half-combine for heads [h0, h0+4): RNn rows
            RNn = big.tile([HD, Q], f32)
            TnF = big.tile([HD, Q], f32)
            M1F = big.tile([HD, Q], f32)

            def emit_num_half(h0):
                rs = slice(h0 * D, h0 * D + 4 * D)  # 32 numerator rows
                nc.vector.tensor_sub(out=TnF[rs, :], in0=UAn[rs, :], in1=U2n[rs, :])
                nc.vector.tensor_mul(out=M1F[rs, :], in0=E1n[rs, :], in1=U1n[rs, :])
                nc.vector.tensor_mul(out=TnF[rs, :], in0=E2n[rs, :], in1=TnF[rs, :])
                nc.vector.tensor_add(out=RNn[rs, :], in0=M1F[rs, :], in1=TnF[rs, :])

            for h in range(4):
                emit_head(h)
            emit_ua2()
            emit_num_half(0)
            for h in range(4, H):
                emit_head(h)
            emit_num_half(4)

            # ---- tail: denominator, reciprocal, normalize, bias, relu
            Td = tmp.tile([H, Q], f32, tag="Td", bufs=1)
            M1d = tmp.tile([H, Q], f32, tag="M1d", bufs=1)
            RNd = tmp.tile([H, Q], f32, tag="RNd", bufs=1)
            nc.vector.tensor_sub(out=Td[:], in0=UAden[:], in1=U2den[:])
            nc.vector.tensor_mul(out=M1d[:], in0=E1T[:], in1=U1den[:])
            nc.vector.tensor_mul(out=Td[:], in0=E2T[:], in1=Td[:])
            nc.vector.tensor_add(out=RNd[:], in0=M1d[:], in1=Td[:])
            rcp = tmp.tile([H, Q], f32, tag="rcp", bufs=1)
            nc.vector.reciprocal(rcp[:], RNd[:])
            ps_rc = ps_small.tile([HD, Q], f32, tag="psBC", bufs=1)
            nc.tensor.matmul(ps_rc[:], sel_sb[:], rcp[:], start=True, stop=True)
            sc = tmp.tile([HD, Q], f32, tag="sc", bufs=1)
            nc.vector.tensor_mul(out=sc[:], in0=RNn[:], in1=ps_rc[:])
            nc.scalar.activation(OUT_T[:], sc[:], Act.Relu, bias=bias_c[:])
            nc.sync.dma_start(out=out_d[:], in_=OUT_T[:])

    if do_compile:
        nc.compile()
    return nc


def _get_nc():
    global _CACHED_NC
    if _CACHED_NC is None:
        _CACHED_NC = build_bass()
    return _CACHED_NC


def make_in_maps(X, A, W, att_self, att_neigh, bias):
    X = np.asarray(X, np.float32)
    A = np.asarray(A, np.float32)
    W = np.asarray(W, np.float32)
    att_self = np.asarray(att_self, np.float32)
    att_neigh = np.asarray(att_neigh, np.float32)
    bias = np.asarray(bias, np.float32)

    XT = np.ascontiguousarray(X.T).astype(ml_dtypes.bfloat16)
    Wbf = W.astype(ml_dtypes.bfloat16)
    attS = np.ascontiguousarray(att_self.reshape(1, HD))
    attN = np.ascontiguousarray(att_neigh.reshape(1, HD))
    bias_c = np.ascontiguousarray(bias.reshape(HD, 1))
    sel = np.zeros((H, HD), np.float32)
    for h in range(H):
        sel[h, h * D : (h + 1) * D] = 1.0
    delta = np.zeros((H, H * 128), np.float32)
    for h in range(H):
        delta[h, h * 128 : (h + 1) * 128] = 1.0
    delta = delta.astype(ml_dtypes.bfloat16)
    in_maps = []
    for c in range(NCORES):
        q0 = c * Q
        AT = np.ascontiguousarray(A[q0 : q0 + Q, :].T).astype(ml_dtypes.bfloat16)
        XqT = np.ascontiguousarray(X[q0 : q0 + Q, :].T)
        in_maps.append(
            {
                "XT": XT,
                "XqT": XqT,
                "W": W,
                "Wbf": Wbf,
                "attS": attS,
                "attN": attN,
                "bias": bias_c,
                "SEL64": sel,
                "DELTA": delta,
                "AT": AT,
            }
        )
    return in_maps


def kernel(X, A, W, att_self, att_neigh, bias, _trace=False, _tmpdir=None):
    from concourse.bass_utils import run_bass_kernel_spmd

    nc = _get_nc()
    in_maps = make_in_maps(X, A, W, att_self, att_neigh, bias)
    res = run_bass_kernel_spmd(
        nc,
        in_maps,
        core_ids=list(range(NCORES)),
        trace=_trace,
        tmpdir=_tmpdir,
    )
    out = np.empty((N, HD), np.float32)
    for c in range(NCORES):
        out[c * Q : (c + 1) * Q, :] = res.results[c]["out"].T
    if _trace:
        return out, res
    return out


# revision 37
# speedup vs baseline: 1.0785x; 1.0390x over previous
"""GAT layer (N=4096, F=64, H=8, D=8) on 8 Trainium2 NeuronCores.

Row-parallel sharding: core c owns queries q0=512*c .. q0+512. Each core
reads the full X (replicated) and the transposed adjacency slice
A_T[j, i] = A[q0+i, j] (host-sliced, cast to bf16 -- exact for a 0/1 mask).

Math: with per-node logits a_s[i,h], a_n[j,h] and s = a_s+a_n,
  exp(leaky_relu(s)) = E1_i*F1_j  if s>=0   (E1=exp(a_s), F1=exp(a_n))
                     = E2_i*F2_j  if s<0    (E2=exp(.2 a_s), F2=exp(.2 a_n))
so no exp over the [H,N,N] tensor is ever needed.  Per head only the {0,1}
mask W1 = A ⊙ [s>=0] is materialized, and the aggregation is PE matmuls:
  U1 = G1^T@W1, U2 = G2^T@W1, UA2 = G2^T@A   (G_k[j] = F_k[j]*[feats_j | 1])
  out = E1*U1 + E2*(UA2-U2);  numerators / denominator; +bias; relu.

Mask production is split across DVE and ACT per 16 key tiles (the Pool
engine rejects TensorTensor/TensorScalarPtr on this toolchain):
  - 7 tiles on DVE: tensor_scalar is_ge ({0,1} sigma, 2x mode) + one
    in-place batched tensor_tensor mult against A (2x mode),
  - 9 tiles on ACT: relu(1e30*u + 1e30*v) -> {0, huge}, then one
    in-place batched DVE min against A.
The UA2 (unmasked) matmul pass runs between heads 3 and 4 when the PE is
warm; E tables are built in the prelude, the numerator combine runs per
head-half (overlapping phase 2), and only den/reciprocal/normalize
remain in the tail. X/W are cast to bf16 on the host for the key-side
feats (the query-side a_s path stays f32).
"""

import sys

sys.path.insert(0, "/opt/trn_rl_repo")

import ml_dtypes
import numpy as np

N, F, H, D = 4096, 64, 8, 8
HD = H * D  # 64
NCORES = 8
Q = N // NCORES  # 512 queries per core
NT = N // 128  # 32 key tiles
QT = Q // 128  # 4 query tiles
GB = 8  # key-tile group size for feats/G-table phase

_CACHED_NC = None


def build_bass(do_compile=True):
    import concourse.bacc as bacc
    import concourse.mybir as mybir
    from concourse.masks import make_identity
    from concourse.tile import TileContext

    f32 = mybir.dt.float32
    bf16 = mybir.dt.bfloat16
    Alu = mybir.AluOpType
    Act = mybir.ActivationFunctionType

    nc = bacc.Bacc()

    XT_d = nc.declare_dram_parameter("XT", [F, N], bf16, isOutput=False)
    XqT_d = nc.declare_dram_parameter("XqT", [F, Q], f32, isOutput=False)
    Wbf_d = nc.declare_dram_parameter("Wbf", [F, HD], bf16, isOutput=False)
    WS_d = nc.declare_dram_parameter("WS", [F, H], f32, isOutput=False)
    attN_d = nc.declare_dram_parameter("attN", [1, HD], f32, isOutput=False)
    bias_d = nc.declare_dram_parameter("bias", [HD, 1], f32, isOutput=False)
    # SEL64[h, h*8+d] = 1 (row -> 8-row group expander)
    sel_d = nc.declare_dram_parameter("SEL64", [H, HD], f32, isOutput=False)
    # DELTA[k, h*128 + p] = (k == h) (row-h selector for K=8 broadcast matmuls)
    delta_d = nc.declare_dram_parameter("DELTA", [H, H * 128], bf16, isOutput=False)
    AT_d = nc.declare_dram_parameter("AT", [N, Q], bf16, isOutput=False)
    out_d = nc.declare_dram_parameter("out", [HD, Q], f32, isOutput=True)

    with TileContext(nc) as tc:
        with (
            tc.tile_pool(name="big", bufs=1) as big,
            tc.tile_pool(name="tmp", bufs=2) as tmp,
            tc.tile_pool(name="w1p", bufs=4) as w1p,
            tc.tile_pool(name="ps_small", bufs=2, space="PSUM") as ps_small,
            tc.tile_pool(name="ps_acc", bufs=1, space="PSUM") as ps_acc,
        ):
            # ---- persistent SBUF tensors
            A_sb = big.tile([128, NT, Q], bf16)  # A_T tiles, [j_tile, i]
            GG = big.tile([128, NT, H * 18], bf16)  # per head: 9 G1 | 9 G2 cols
            G2a = big.tile([128, NT, 72], bf16)  # contiguous all-heads G2
            a_sbB = big.tile([128, H, Q], bf16)  # a_s bcast over partitions
            na_all = big.tile([128, NT, H], f32)  # -a_n
            na_big = big.tile([128, NT, H], f32)  # +1e30 * a_n
            feats_all = big.tile([128, NT, HD], bf16)
            F1_all = big.tile([128, NT, H], f32)
            F2_all = big.tile([128, NT, H], f32)
            XT_sb = big.tile([F, N], bf16)
            XqT_sb = big.tile([F, Q], f32)
            Wbf_sb = big.tile([F, HD], bf16)
            WS_sb = big.tile([F, H], f32)
            attN_b = big.tile([128, HD], bf16)
            bias_c = big.tile([HD, 1], f32)
            sel_sb = big.tile([H, HD], f32)
            delta_sb = big.tile([H, H * 128], bf16)
            a_sT = big.tile([H, Q], f32)
            a_sT_bf = big.tile([H, Q], bf16)
            E1T = big.tile([H, Q], f32)
            E2T = big.tile([H, Q], f32)
            E1n = big.tile([HD, Q], f32)
            E2n = big.tile([HD, Q], f32)
            U1n = big.tile([HD, Q], f32)
            U2n = big.tile([HD, Q], f32)
            UAn = big.tile([HD, Q], f32)
            U1den = big.tile([H, Q], f32)
            U2den = big.tile([H, Q], f32)
            UAden = big.tile([H, Q], f32)
            OUT_T = big.tile([HD, Q], f32)

            # ---- input DMAs (small phase-0-critical tensors FIRST)
            nc.sync.dma_start(out=WS_sb[:], in_=WS_d[:])
            nc.sync.dma_start(out=XqT_sb[:], in_=XqT_d[:])
            nc.sync.dma_start(out=XT_sb[:], in_=XT_d[:])
            nc.sync.dma_start(out=Wbf_sb[:], in_=Wbf_d[:])
            nc.sync.dma_start(out=bias_c[:], in_=bias_d[:])
            nc.sync.dma_start(out=sel_sb[:], in_=sel_d[:])
            nc.sync.dma_start(out=delta_sb[:], in_=delta_d[:])
            att_row2 = tmp.tile([1, HD], f32, tag="attrow")
            nc.sync.dma_start(out=att_row2[:], in_=attN_d[:])
            for nt in range(NT):
                nc.sync.dma_start(
                    out=A_sb[:, nt, :], in_=AT_d[nt * 128 : (nt + 1) * 128, :]
                )

            # broadcast helper: ones row for K=1 "broadcast" matmuls
            ones_f = big.tile([1, 128], f32)
            nc.gpsimd.memset(ones_f[:], 1.0)
            ps_b2 = ps_small.tile([128, HD], f32, tag="psB", bufs=1)
            nc.tensor.matmul(ps_b2[:], ones_f[:], att_row2[:], start=True, stop=True)
            nc.scalar.copy(attN_b[:], ps_b2[:])

            # ---- a_s for this core's queries: a_sT = WS^T @ XqT directly
            psT8 = ps_small.tile([H, Q], f32, tag="psT", bufs=1)
            nc.tensor.matmul(psT8[:], WS_sb[:], XqT_sb[:], start=True, stop=True)
            nc.scalar.copy(a_sT[:], psT8[:])

            nc.vector.tensor_copy(out=a_sT_bf[:], in_=a_sT[:])
            # a_sbB[:, h, :] = row h of a_sT broadcast to 128 partitions
            for h in range(H):
                ps_bc = ps_small.tile([128, Q], f32, tag="psBC", bufs=1)
                nc.tensor.matmul(
                    ps_bc[:],
                    delta_sb[:, h * 128 : (h + 1) * 128],
                    a_sT_bf[:],
                    start=True,
                    stop=True,
                )
                nc.scalar.copy(a_sbB[:, h, :], ps_bc[:])

            # E tables (needed only by the combine; PE/ACT have slack here)
            nc.scalar.activation(E1T[:], a_sT[:], Act.Exp)
            nc.scalar.activation(E2T[:], a_sT[:], Act.Exp, scale=0.2)
            ps_e = ps_small.tile([HD, Q], f32, tag="psBC", bufs=1)
            nc.tensor.matmul(ps_e[:], sel_sb[:], E1T[:], start=True, stop=True)
            nc.vector.tensor_copy(out=E1n[:], in_=ps_e[:])
            ps_e2 = ps_small.tile([HD, Q], f32, tag="psBC", bufs=1)
            nc.tensor.matmul(ps_e2[:], sel_sb[:], E2T[:], start=True, stop=True)
            nc.scalar.copy(E2n[:], ps_e2[:])

            # ---- phase 0: feats, a_n, F1/F2, G tables (all 4096 nodes)
            gg_all = GG.rearrange("p a (h u) -> p a h u", u=18)
            g2_all = G2a.rearrange("p a (h u) -> p a h u", u=9)
            feats_v = feats_all.rearrange("p a (h d) -> p a h d", d=D)
            for g in range(0, NT, GB):
                sl = slice(g, g + GB)
                psF = ps_small.tile([128, GB * HD], f32, tag="psF")
                for t in range(GB):
                    nt = g + t
                    nc.tensor.matmul(
                        psF[:, t * HD : (t + 1) * HD],
                        XT_sb[:, nt * 128 : (nt + 1) * 128],
                        Wbf_sb[:],
                        start=True,
                        stop=True,
                    )
                nc.scalar.copy(
                    feats_all[:, sl, :].rearrange("p a b -> p (a b)"),
                    psF[:],
                )
                prod = tmp.tile([128, GB, HD], bf16, tag="prod")
                nc.vector.tensor_tensor(
                    out=prod[:],
                    in0=feats_all[:, sl, :],
                    in1=attN_b[:, None, :].broadcast_to([128, GB, HD]),
                    op=Alu.mult,
                )
                nc.vector.tensor_reduce(
                    out=na_all[:, sl, :],
                    in_=prod.rearrange("p a (h d) -> p a h d", d=D),
                    axis=mybir.AxisListType.X,
                    op=Alu.add,
                    negate=True,
                )
                nc.vector.tensor_scalar_mul(
                    out=na_big[:, sl, :].rearrange("p a b -> p (a b)"),
                    in0=na_all[:, sl, :].rearrange("p a b -> p (a b)"),
                    scalar1=-1e30,
                )
                nc.scalar.activation(
                    F1_all[:, sl, :].rearrange("p a b -> p (a b)"),
                    na_all[:, sl, :].rearrange("p a b -> p (a b)"),
                    Act.Exp,
                    scale=-1.0,
                )
                nc.scalar.activation(
                    F2_all[:, sl, :].rearrange("p a b -> p (a b)"),
                    na_all[:, sl, :].rearrange("p a b -> p (a b)"),
                    Act.Exp,
                    scale=-0.2,
                )
                nc.vector.tensor_tensor(
                    out=gg_all[:, sl, :, 0:8],
                    in0=feats_v[:, sl],
                    in1=F1_all[:, sl, :, None].broadcast_to([128, GB, H, D]),
                    op=Alu.mult,
                )
                nc.vector.tensor_copy(
                    out=gg_all[:, sl, :, 8:9], in_=F1_all[:, sl, :, None]
                )
                nc.vector.tensor_tensor(
                    out=g2_all[:, sl, :, 0:8],
                    in0=feats_v[:, sl],
                    in1=F2_all[:, sl, :, None].broadcast_to([128, GB, H, D]),
                    op=Alu.mult,
                )
                nc.vector.tensor_copy(
                    out=g2_all[:, sl, :, 8:9], in_=F2_all[:, sl, :, None]
                )
                nc.vector.tensor_copy(
                    out=gg_all[:, sl, :, 9:18], in_=g2_all[:, sl]
                )

            # ---- phase 2: per head, W1 = [s>=0] ⊙ A; U = GG_h^T @ W1
            # per 16-tile block: 6 DVE / 5 ACT / 5 Pool mask tiles
            def emit_head(h):
                psU = ps_acc.tile([18, Q], f32, tag="psU", bufs=2)
                for b in range(0, NT, 16):
                    # DVE tiles b..b+6: sigma in {0,1} via 4x tensor_scalar,
                    # then one in-place batched mult against A
                    w1D = w1p.tile([128, 7, Q], bf16, tag="w1D")
                    for t in range(7):
                        nt = b + t
                        nc.vector.tensor_scalar(
                            out=w1D[:, t, :],
                            in0=a_sbB[:, h, :],
                            scalar1=na_all[:, nt, h : h + 1],
                            scalar2=None,
                            op0=Alu.is_ge,
                        )
                    nc.vector.tensor_tensor(
                        out=w1D[:],
                        in0=w1D[:],
                        in1=A_sb[:, b : b + 7, :],
                        op=Alu.mult,
                    )
                    # ACT tiles b+7..b+15: sigma in {0, huge} via relu,
                    # then one in-place batched DVE min against A
                    w1A = w1p.tile([128, 9, Q], bf16, tag="w1A")
                    for t in range(9):
                        nt = b + 7 + t
                        nc.scalar.activation(
                            w1A[:, t, :],
                            a_sbB[:, h, :],
                            Act.Relu,
                            scale=1e30,
                            bias=na_big[:, nt, h : h + 1],
                        )
                    nc.vector.tensor_tensor(
                        out=w1A[:, 0:5, :],
                        in0=w1A[:, 0:5, :],
                        in1=A_sb[:, b + 7 : b + 12, :],
                        op=Alu.min,
                    )
                    nc.vector.tensor_tensor(
                        out=w1A[:, 5:9, :],
                        in0=w1A[:, 5:9, :],
                        in1=A_sb[:, b + 12 : b + 16, :],
                        op=Alu.min,
                    )
                    # matmuls in nt order
                    for t in range(16):
                        nt = b + t
                        if t < 7:
                            rhs = w1D[:, t, :]
                        else:
                            rhs = w1A[:, t - 7, :]
                        nc.tensor.matmul(
                            psU[:],
                            GG[:, nt, h * 18 : (h + 1) * 18],
                            rhs,
                            start=(nt == 0),
                            stop=(nt == NT - 1),
                        )
                U_sb = tmp.tile([18, Q], f32, tag="U", bufs=3)
                if h % 2 == 0:
                    nc.scalar.copy(U_sb[:], psU[:])
                else:
                    nc.vector.tensor_copy(out=U_sb[:], in_=psU[:])
                nc.sync.dma_start(
                    out=U1n[h * D : (h + 1) * D, :], in_=U_sb[0:8, :]
                )
                nc.sync.dma_start(out=U1den[h : h + 1, :], in_=U_sb[8:9, :])
                nc.sync.dma_start(
                    out=U2n[h * D : (h + 1) * D, :], in_=U_sb[9:17, :]
                )
                nc.sync.dma_start(out=U2den[h : h + 1, :], in_=U_sb[17:18, :])

            def emit_ua2():
                # UA2 = G2^T @ A_T -> [72, Q], strided G2 columns from GG
                psA = ps_acc.tile([72, Q], f32, tag="psA")
                for nt in range(NT):
                    nc.tensor.matmul(
                        psA[:],
                        G2a[:, nt, :],
                        A_sb[:, nt, :],
                        start=(nt == 0),
                        stop=(nt == NT - 1),
                    )
                UA_sb = tmp.tile([72, Q], f32, tag="UA", bufs=1)
                nc.vector.tensor_copy(out=UA_sb[:], in_=psA[:])
                for h in range(H):
                    nc.sync.dma_start(
                        out=UAn[h * D : (h + 1) * D, :],
                        in_=UA_sb[h * 9 : h * 9 + 8, :],
                    )
                    nc.sync.dma_start(
                        out=UAden[h : h + 1, :],
                        in_=UA_sb[h * 9 + 8 : h * 9 + 9, :],
                    )

            # numerator half-combine for heads [h0, h0+4): RNn rows
            RNn = big.tile([HD, Q], f32)
            TnF = big.tile([HD, Q], f32)
            M1F = big.tile([HD, Q], f32)

            def emit_num_half(h0):
                rs = slice(h0 * D, h0 * D + 4 * D)  # 32 numerator rows
                nc.vector.tensor_sub(out=TnF[rs, :], in0=UAn[rs, :], in1=U2n[rs, :])
                nc.vector.tensor_mul(out=M1F[rs, :], in0=E1n[rs, :], in1=U1n[rs, :])
                nc.vector.tensor_mul(out=TnF[rs, :], in0=E2n[rs, :], in1=TnF[rs, :])
                nc.vector.tensor_add(out=RNn[rs, :], in0=M1F[rs, :], in1=TnF[rs, :])

            for h in range(4):
                emit_head(h)
            emit_ua2()
            emit_num_half(0)
            for h in range(4, H):
                emit_head(h)
            emit_num_half(4)

            # ---- tail: denominator, reciprocal, normalize, bias, relu
            Td = tmp.tile([H, Q], f32, tag="Td", bufs=1)
            M1d = tmp.tile([H, Q], f32, tag="M1d", bufs=1)
            RNd = tmp.tile([H, Q], f32, tag="RNd", bufs=1)
            nc.vector.tensor_sub(out=Td[:], in0=UAden[:], in1=U2den[:])
            nc.vector.tensor_mul(out=M1d[:], in0=E1T[:], in1=U1den[:])
            nc.vector.tensor_mul(out=Td[:], in0=E2T[:], in1=Td[:])
            nc.vector.tensor_add(out=RNd[:], in0=M1d[:], in1=Td[:])
            rcp = tmp.tile([H, Q], f32, tag="rcp", bufs=1)
            nc.vector.reciprocal(rcp[:], RNd[:])
            ps_rc = ps_small.tile([HD, Q], f32, tag="psBC", bufs=1)
            nc.tensor.matmul(ps_rc[:], sel_sb[:], rcp[:], start=True, stop=True)
            sc = tmp.tile([HD, Q], f32, tag="sc", bufs=1)
            nc.vector.tensor_mul(out=sc[:], in0=RNn[:], in1=ps_rc[:])
            nc.scalar.activation(OUT_T[:], sc[:], Act.Relu, bias=bias_c[:])
            nc.sync.dma_start(out=out_d[:], in_=OUT_T[:])

    if do_compile:
        nc.compile()
    return nc


def _get_nc():
    global _CACHED_NC
    if _CACHED_NC is None:
        _CACHED_NC = build_bass()
    return _CACHED_NC


def make_in_maps(X, A, W, att_self, att_neigh, bias):
    X = np.asarray(X, np.float32)
    A = np.asarray(A, np.float32)
    W = np.asarray(W, np.float32)
    att_self = np.asarray(att_self, np.float32)
    att_neigh = np.asarray(att_neigh, np.float32)
    bias = np.asarray(bias, np.float32)

    XT = np.ascontiguousarray(X.T).astype(ml_dtypes.bfloat16)
    Wbf = W.astype(ml_dtypes.bfloat16)
    WS = np.ascontiguousarray(
        np.einsum("fhd,hd->fh", W.reshape(F, H, D), att_self).astype(np.float32)
    )
    attN = np.ascontiguousarray(att_neigh.reshape(1, HD))
    bias_c = np.ascontiguousarray(bias.reshape(HD, 1))
    sel = np.zeros((H, HD), np.float32)
    for h in range(H):
        sel[h, h * D : (h + 1) * D] = 1.0
    delta = np.zeros((H, H * 128), np.float32)
    for h in range(H):
        delta[h, h * 128 : (h + 1) * 128] = 1.0
    delta = delta.astype(ml_dtypes.bfloat16)
    in_maps = []
    for c in range(NCORES):
        q0 = c * Q
        AT = np.ascontiguousarray(A[q0 : q0 + Q, :].T).astype(ml_dtypes.bfloat16)
        XqT = np.ascontiguousarray(X[q0 : q0 + Q, :].T)
        in_maps.append(
            {
                "XT": XT,
                "XqT": XqT,
                "Wbf": Wbf,
                "WS": WS,
                "attN": attN,
                "bias": bias_c,
                "SEL64": sel,
                "DELTA": delta,
                "AT": AT,
            }
        )
    return in_maps


def kernel(X, A, W, att_self, att_neigh, bias, _trace=False, _tmpdir=None):
    from concourse.bass_utils import run_bass_kernel_spmd

    nc = _get_nc()
    in_maps = make_in_maps(X, A, W, att_self, att_neigh, bias)
    res = run_bass_kernel_spmd(
        nc,
        in_maps,
        core_ids=list(range(NCORES)),
        trace=_trace,
        tmpdir=_tmpdir,
    )
    out = np.empty((N, HD), np.float32)
    for c in range(NCORES):
        out[c * Q : (c + 1) * Q, :] = res.results[c]["out"].T
    if _trace:
        return out, res
    return out


# revision 38
# speedup vs baseline: 1.0956x; 1.0158x over previous
"""GAT layer (N=4096, F=64, H=8, D=8) on 8 Trainium2 NeuronCores.

Row-parallel sharding: core c owns queries q0=512*c .. q0+512. Each core
reads the full X (replicated) and the transposed adjacency slice
A_T[j, i] = A[q0+i, j] (host-sliced, cast to bf16 -- exact for a 0/1 mask).

Math: with per-node logits a_s[i,h], a_n[j,h] and s = a_s+a_n,
  exp(leaky_relu(s)) = E1_i*F1_j  if s>=0   (E1=exp(a_s), F1=exp(a_n))
                     = E2_i*F2_j  if s<0    (E2=exp(.2 a_s), F2=exp(.2 a_n))
so no exp over the [H,N,N] tensor is ever needed.  Per head only the {0,1}
mask W1 = A ⊙ [s>=0] is materialized, and the aggregation is PE matmuls:
  U1 = G1^T@W1, U2 = G2^T@W1, UA2 = G2^T@A   (G_k[j] = F_k[j]*[feats_j | 1])
  out = E1*U1 + E2*(UA2-U2);  numerators / denominator; +bias; relu.

Mask production is split across DVE and ACT per 16 key tiles (the Pool
engine rejects TensorTensor/TensorScalarPtr on this toolchain):
  - 6 tiles on DVE: tensor_scalar is_ge ({0,1} sigma, 2x mode) + one
    in-place batched tensor_tensor mult against A (2x mode),
  - 10 tiles on ACT: relu(1e30*u + 1e30*v) -> {0, huge}, then one
    in-place batched DVE min against A.
The UA2 (unmasked) matmul pass runs between heads 3 and 4 when the PE is
warm; E tables are built in the prelude, the numerator combine runs per
head-half (overlapping phase 2), and only den/reciprocal/normalize
remain in the tail. X/W are cast to bf16 on the host for the key-side
feats (the query-side a_s path stays f32).
"""

import sys

sys.path.insert(0, "/opt/trn_rl_repo")

import ml_dtypes
import numpy as np

N, F, H, D = 4096, 64, 8, 8
HD = H * D  # 64
NCORES = 8
Q = N // NCORES  # 512 queries per core
NT = N // 128  # 32 key tiles
QT = Q // 128  # 4 query tiles
GB = 8  # key-tile group size for feats/G-table phase

_CACHED_NC = None


def build_bass(do_compile=True):
    import concourse.bacc as bacc
    import concourse.mybir as mybir
    from concourse.masks import make_identity
    from concourse.tile import TileContext

    f32 = mybir.dt.float32
    bf16 = mybir.dt.bfloat16
    Alu = mybir.AluOpType
    Act = mybir.ActivationFunctionType

    nc = bacc.Bacc()

    XT_d = nc.declare_dram_parameter("XT", [F, N], bf16, isOutput=False)
    XqT_d = nc.declare_dram_parameter("XqT", [F, Q], f32, isOutput=False)
    Wbf_d = nc.declare_dram_parameter("Wbf", [F, HD], bf16, isOutput=False)
    WS_d = nc.declare_dram_parameter("WS", [F, H], f32, isOutput=False)
    attN_d = nc.declare_dram_parameter("attN", [1, HD], f32, isOutput=False)
    bias_d = nc.declare_dram_parameter("bias", [HD, 1], f32, isOutput=False)
    # SEL64[h, h*8+d] = 1 (row -> 8-row group expander)
    sel_d = nc.declare_dram_parameter("SEL64", [H, HD], f32, isOutput=False)
    # DELTA[k, h*128 + p] = (k == h) (row-h selector for K=8 broadcast matmuls)
    delta_d = nc.declare_dram_parameter("DELTA", [H, H * 128], bf16, isOutput=False)
    AT_d = nc.declare_dram_parameter("AT", [N, Q], bf16, isOutput=False)
    out_d = nc.declare_dram_parameter("out", [HD, Q], f32, isOutput=True)

    with TileContext(nc) as tc:
        with (
            tc.tile_pool(name="big", bufs=1) as big,
            tc.tile_pool(name="tmp", bufs=2) as tmp,
            tc.tile_pool(name="w1p", bufs=4) as w1p,
            tc.tile_pool(name="ps_small", bufs=2, space="PSUM") as ps_small,
            tc.tile_pool(name="ps_acc", bufs=1, space="PSUM") as ps_acc,
        ):
            # ---- persistent SBUF tensors
            A_sb = big.tile([128, NT, Q], bf16)  # A_T tiles, [j_tile, i]
            GG = big.tile([128, NT, H * 18], bf16)  # per head: 9 G1 | 9 G2 cols
            G2a = big.tile([128, NT, 72], bf16)  # contiguous all-heads G2
            a_sbB = big.tile([128, H, Q], bf16)  # a_s bcast over partitions
            na_all = big.tile([128, NT, H], f32)  # -a_n
            na_big = big.tile([128, NT, H], f32)  # +1e30 * a_n
            feats_all = big.tile([128, NT, HD], bf16)
            F1_all = big.tile([128, NT, H], f32)
            F2_all = big.tile([128, NT, H], f32)
            XT_sb = big.tile([F, N], bf16)
            XqT_sb = big.tile([F, Q], f32)
            Wbf_sb = big.tile([F, HD], bf16)
            WS_sb = big.tile([F, H], f32)
            attN_b = big.tile([128, HD], bf16)
            bias_c = big.tile([HD, 1], f32)
            sel_sb = big.tile([H, HD], f32)
            delta_sb = big.tile([H, H * 128], bf16)
            a_sT = big.tile([H, Q], f32)
            a_sT_bf = big.tile([H, Q], bf16)
            E1T = big.tile([H, Q], f32)
            E2T = big.tile([H, Q], f32)
            E1n = big.tile([HD, Q], f32)
            E2n = big.tile([HD, Q], f32)
            U1n = big.tile([HD, Q], f32)
            U2n = big.tile([HD, Q], f32)
            UAn = big.tile([HD, Q], f32)
            U1den = big.tile([H, Q], f32)
            U2den = big.tile([H, Q], f32)
            UAden = big.tile([H, Q], f32)
            OUT_T = big.tile([HD, Q], f32)

            # ---- input DMAs (small phase-0-critical tensors FIRST)
            nc.sync.dma_start(out=WS_sb[:], in_=WS_d[:])
            nc.sync.dma_start(out=XqT_sb[:], in_=XqT_d[:])
            nc.sync.dma_start(out=XT_sb[:], in_=XT_d[:])
            nc.sync.dma_start(out=Wbf_sb[:], in_=Wbf_d[:])
            nc.sync.dma_start(out=bias_c[:], in_=bias_d[:])
            nc.sync.dma_start(out=sel_sb[:], in_=sel_d[:])
            nc.sync.dma_start(out=delta_sb[:], in_=delta_d[:])
            att_row2 = tmp.tile([1, HD], f32, tag="attrow")
            nc.sync.dma_start(out=att_row2[:], in_=attN_d[:])
            for nt in range(NT):
                nc.sync.dma_start(
                    out=A_sb[:, nt, :], in_=AT_d[nt * 128 : (nt + 1) * 128, :]
                )

            # broadcast helper: ones row for K=1 "broadcast" matmuls
            ones_f = big.tile([1, 128], f32)
            nc.gpsimd.memset(ones_f[:], 1.0)
            ps_b2 = ps_small.tile([128, HD], f32, tag="psB", bufs=1)
            nc.tensor.matmul(ps_b2[:], ones_f[:], att_row2[:], start=True, stop=True)
            nc.scalar.copy(attN_b[:], ps_b2[:])

            # ---- a_s for this core's queries: a_sT = WS^T @ XqT directly
            psT8 = ps_small.tile([H, Q], f32, tag="psT", bufs=1)
            nc.tensor.matmul(psT8[:], WS_sb[:], XqT_sb[:], start=True, stop=True)
            nc.scalar.copy(a_sT[:], psT8[:])

            nc.vector.tensor_copy(out=a_sT_bf[:], in_=a_sT[:])
            # a_sbB[:, h, :] = row h of a_sT broadcast to 128 partitions
            for h in range(H):
                ps_bc = ps_small.tile([128, Q], f32, tag="psBC", bufs=1)
                nc.tensor.matmul(
                    ps_bc[:],
                    delta_sb[:, h * 128 : (h + 1) * 128],
                    a_sT_bf[:],
                    start=True,
                    stop=True,
                )
                nc.scalar.copy(a_sbB[:, h, :], ps_bc[:])

            # E tables (needed only by the combine; PE/ACT have slack here)
            nc.scalar.activation(E1T[:], a_sT[:], Act.Exp)
            nc.scalar.activation(E2T[:], a_sT[:], Act.Exp, scale=0.2)
            ps_e = ps_small.tile([HD, Q], f32, tag="psBC", bufs=1)
            nc.tensor.matmul(ps_e[:], sel_sb[:], E1T[:], start=True, stop=True)
            nc.vector.tensor_copy(out=E1n[:], in_=ps_e[:])
            ps_e2 = ps_small.tile([HD, Q], f32, tag="psBC", bufs=1)
            nc.tensor.matmul(ps_e2[:], sel_sb[:], E2T[:], start=True, stop=True)
            nc.scalar.copy(E2n[:], ps_e2[:])

            # ---- phase 0: feats, a_n, F1/F2, G tables (all 4096 nodes)
            gg_all = GG.rearrange("p a (h u) -> p a h u", u=18)
            g2_all = G2a.rearrange("p a (h u) -> p a h u", u=9)
            feats_v = feats_all.rearrange("p a (h d) -> p a h d", d=D)
            for g in range(0, NT, GB):
                sl = slice(g, g + GB)
                psF = ps_small.tile([128, GB * HD], f32, tag="psF")
                for t in range(GB):
                    nt = g + t
                    nc.tensor.matmul(
                        psF[:, t * HD : (t + 1) * HD],
                        XT_sb[:, nt * 128 : (nt + 1) * 128],
                        Wbf_sb[:],
                        start=True,
                        stop=True,
                    )
                nc.scalar.copy(
                    feats_all[:, sl, :].rearrange("p a b -> p (a b)"),
                    psF[:],
                )
                prod = tmp.tile([128, GB, HD], bf16, tag="prod")
                nc.vector.tensor_tensor(
                    out=prod[:],
                    in0=feats_all[:, sl, :],
                    in1=attN_b[:, None, :].broadcast_to([128, GB, HD]),
                    op=Alu.mult,
                )
                nc.vector.tensor_reduce(
                    out=na_all[:, sl, :],
                    in_=prod.rearrange("p a (h d) -> p a h d", d=D),
                    axis=mybir.AxisListType.X,
                    op=Alu.add,
                    negate=True,
                )
                nc.vector.tensor_scalar_mul(
                    out=na_big[:, sl, :].rearrange("p a b -> p (a b)"),
                    in0=na_all[:, sl, :].rearrange("p a b -> p (a b)"),
                    scalar1=-1e30,
                )
                nc.scalar.activation(
                    F1_all[:, sl, :].rearrange("p a b -> p (a b)"),
                    na_all[:, sl, :].rearrange("p a b -> p (a b)"),
                    Act.Exp,
                    scale=-1.0,
                )
                nc.scalar.activation(
                    F2_all[:, sl, :].rearrange("p a b -> p (a b)"),
                    na_all[:, sl, :].rearrange("p a b -> p (a b)"),
                    Act.Exp,
                    scale=-0.2,
                )
                nc.vector.tensor_tensor(
                    out=gg_all[:, sl, :, 0:8],
                    in0=feats_v[:, sl],
                    in1=F1_all[:, sl, :, None].broadcast_to([128, GB, H, D]),
                    op=Alu.mult,
                )
                nc.vector.tensor_copy(
                    out=gg_all[:, sl, :, 8:9], in_=F1_all[:, sl, :, None]
                )
                nc.vector.tensor_tensor(
                    out=g2_all[:, sl, :, 0:8],
                    in0=feats_v[:, sl],
                    in1=F2_all[:, sl, :, None].broadcast_to([128, GB, H, D]),
                    op=Alu.mult,
                )
                nc.vector.tensor_copy(
                    out=g2_all[:, sl, :, 8:9], in_=F2_all[:, sl, :, None]
                )
                nc.vector.tensor_copy(
                    out=gg_all[:, sl, :, 9:18], in_=g2_all[:, sl]
                )

            # ---- phase 2: per head, W1 = [s>=0] ⊙ A; U = GG_h^T @ W1
            # per 16-tile block: 6 DVE / 5 ACT / 5 Pool mask tiles
            def emit_head(h):
                psU = ps_acc.tile([18, Q], f32, tag="psU", bufs=2)
                for b in range(0, NT, 16):
                    # DVE tiles b..b+5: sigma in {0,1} via tensor_scalar,
                    # then one in-place batched mult against A
                    w1D = w1p.tile([128, 6, Q], bf16, tag="w1D")
                    for t in range(6):
                        nt = b + t
                        nc.vector.tensor_scalar(
                            out=w1D[:, t, :],
                            in0=a_sbB[:, h, :],
                            scalar1=na_all[:, nt, h : h + 1],
                            scalar2=None,
                            op0=Alu.is_ge,
                        )
                    nc.vector.tensor_tensor(
                        out=w1D[:],
                        in0=w1D[:],
                        in1=A_sb[:, b : b + 6, :],
                        op=Alu.mult,
                    )
                    # ACT tiles b+6..b+15: sigma in {0, huge} via relu,
                    # then one in-place batched DVE min against A
                    w1A = w1p.tile([128, 10, Q], bf16, tag="w1A")
                    for t in range(10):
                        nt = b + 6 + t
                        nc.scalar.activation(
                            w1A[:, t, :],
                            a_sbB[:, h, :],
                            Act.Relu,
                            scale=1e30,
                            bias=na_big[:, nt, h : h + 1],
                        )
                    nc.vector.tensor_tensor(
                        out=w1A[:, 0:5, :],
                        in0=w1A[:, 0:5, :],
                        in1=A_sb[:, b + 6 : b + 11, :],
                        op=Alu.min,
                    )
                    nc.vector.tensor_tensor(
                        out=w1A[:, 5:10, :],
                        in0=w1A[:, 5:10, :],
                        in1=A_sb[:, b + 11 : b + 16, :],
                        op=Alu.min,
                    )
                    # matmuls in nt order
                    for t in range(16):
                        nt = b + t
                        if t < 6:
                            rhs = w1D[:, t, :]
                        else:
                            rhs = w1A[:, t - 6, :]
                        nc.tensor.matmul(
                            psU[:],
                            GG[:, nt, h * 18 : (h + 1) * 18],
                            rhs,
                            start=(nt == 0),
                            stop=(nt == NT - 1),
                        )
                U_sb = tmp.tile([18, Q], f32, tag="U", bufs=3)
                if h % 2 == 0:
                    nc.scalar.copy(U_sb[:], psU[:])
                else:
                    nc.vector.tensor_copy(out=U_sb[:], in_=psU[:])
                nc.sync.dma_start(
                    out=U1n[h * D : (h + 1) * D, :], in_=U_sb[0:8, :]
                )
                nc.sync.dma_start(out=U1den[h : h + 1, :], in_=U_sb[8:9, :])
                nc.sync.dma_start(
                    out=U2n[h * D : (h + 1) * D, :], in_=U_sb[9:17, :]
                )
                nc.sync.dma_start(out=U2den[h : h + 1, :], in_=U_sb[17:18, :])

            def emit_ua2():
                # UA2 = G2^T @ A_T -> [72, Q], strided G2 columns from GG
                psA = ps_acc.tile([72, Q], f32, tag="psA")
                for nt in range(NT):
                    nc.tensor.matmul(
                        psA[:],
                        G2a[:, nt, :],
                        A_sb[:, nt, :],
                        start=(nt == 0),
                        stop=(nt == NT - 1),
                    )
                UA_sb = tmp.tile([72, Q], f32, tag="UA", bufs=1)
                nc.vector.tensor_copy(out=UA_sb[:], in_=psA[:])
                for h in range(H):
                    nc.sync.dma_start(
                        out=UAn[h * D : (h + 1) * D, :],
                        in_=UA_sb[h * 9 : h * 9 + 8, :],
                    )
                    nc.sync.dma_start(
                        out=UAden[h : h + 1, :],
                        in_=UA_sb[h * 9 + 8 : h * 9 + 9, :],
                    )

            # numerator half-combine for heads [h0, h0+4): RNn rows
            RNn = big.tile([HD, Q], f32)
            TnF = big.tile([HD, Q], f32)
            M1F = big.tile([HD, Q], f32)

            def emit_num_half(h0):
                rs = slice(h0 * D, h0 * D + 4 * D)  # 32 numerator rows
                nc.vector.tensor_sub(out=TnF[rs, :], in0=UAn[rs, :], in1=U2n[rs, :])
                nc.vector.tensor_mul(out=M1F[rs, :], in0=E1n[rs, :], in1=U1n[rs, :])
                nc.vector.tensor_mul(out=TnF[rs, :], in0=E2n[rs, :], in1=TnF[rs, :])
                nc.vector.tensor_add(out=RNn[rs, :], in0=M1F[rs, :], in1=TnF[rs, :])

            for h in range(4):
                emit_head(h)
            emit_ua2()
            emit_num_half(0)
            for h in range(4, H):
                emit_head(h)
            emit_num_half(4)

            # ---- tail: denominator, reciprocal, normalize, bias, relu
            Td = tmp.tile([H, Q], f32, tag="Td", bufs=1)
            M1d = tmp.tile([H, Q], f32, tag="M1d", bufs=1)
            RNd = tmp.tile([H, Q], f32, tag="RNd", bufs=1)
            nc.vector.tensor_sub(out=Td[:], in0=UAden[:], in1=U2den[:])
            nc.vector.tensor_mul(out=M1d[:], in0=E1T[:], in1=U1den[:])
            nc.vector.tensor_mul(out=Td[:], in0=E2T[:], in1=Td[:])
            nc.vector.tensor_add(out=RNd[:], in0=M1d[:], in1=Td[:])
            rcp = tmp.tile([H, Q], f32, tag="rcp", bufs=1)
            nc.vector.reciprocal(rcp[:], RNd[:])
            ps_rc = ps_small.tile([HD, Q], f32, tag="psBC", bufs=1)
            nc.tensor.matmul(ps_rc[:], sel_sb[:], rcp[:], start=True, stop=True)
            sc = tmp.tile([HD, Q], f32, tag="sc", bufs=1)
            nc.vector.tensor_mul(out=sc[:], in0=RNn[:], in1=ps_rc[:])
            nc.scalar.activation(OUT_T[:], sc[:], Act.Relu, bias=bias_c[:])
            nc.sync.dma_start(out=out_d[:], in_=OUT_T[:])

    if do_compile:
        nc.compile()
    return nc


def _get_nc():
    global _CACHED_NC
    if _CACHED_NC is None:
        _CACHED_NC = build_bass()
    return _CACHED_NC


def make_in_maps(X, A, W, att_self, att_neigh, bias):
    X = np.asarray(X, np.float32)
    A = np.asarray(A, np.float32)
    W = np.asarray(W, np.float32)
    att_self = np.asarray(att_self, np.float32)
    att_neigh = np.asarray(att_neigh, np.float32)
    bias = np.asarray(bias, np.float32)

    XT = np.ascontiguousarray(X.T).astype(ml_dtypes.bfloat16)
    Wbf = W.astype(ml_dtypes.bfloat16)
    WS = np.ascontiguousarray(
        np.einsum("fhd,hd->fh", W.reshape(F, H, D), att_self).astype(np.float32)
    )
    attN = np.ascontiguousarray(att_neigh.reshape(1, HD))
    bias_c = np.ascontiguousarray(bias.reshape(HD, 1))
    sel = np.zeros((H, HD), np.float32)
    for h in range(H):
        sel[h, h * D : (h + 1) * D] = 1.0
    delta = np.zeros((H, H * 128), np.float32)
    for h in range(H):
        delta[h, h * 128 : (h + 1) * 128] = 1.0
    delta = delta.astype(ml_dtypes.bfloat16)
    in_maps = []
    for c in range(NCORES):
        q0 = c * Q
        AT = np.ascontiguousarray(A[q0 : q0 + Q, :].T).astype(ml_dtypes.bfloat16)
        XqT = np.ascontiguousarray(X[q0 : q0 + Q, :].T)
        in_maps.append(
            {
                "XT": XT,
                "XqT": XqT,
                "Wbf": Wbf,
                "WS": WS,
                "attN": attN,
                "bias": bias_c,
                "SEL64": sel,
                "DELTA": delta,
                "AT": AT,
            }
        )
    return in_maps


def kernel(X, A, W, att_self, att_neigh, bias, _trace=False, _tmpdir=None):
    from concourse.bass_utils import run_bass_kernel_spmd

    nc = _get_nc()
    in_maps = make_in_maps(X, A, W, att_self, att_neigh, bias)
    res = run_bass_kernel_spmd(
        nc,
        in_maps,
        core_ids=list(range(NCORES)),
        trace=_trace,
        tmpdir=_tmpdir,
    )
    out = np.empty((N, HD), np.float32)
    for c in range(NCORES):
        out[c * Q : (c + 1) * Q, :] = res.results[c]["out"].T
    if _trace:
        return out, res
    return out
